# revision 26
# baseline (speedup 1.0000x reference)
"""Causal attention kernel for Trainium2, 8 NeuronCores, sequence-parallel.

Reference computation (T=4096, D=1024, fp32):
    q = x @ Wqk; logits = q @ x.T (causal masked); attn = softmax(logits)
    out = (attn @ x) @ Wov

Sharding: query rows split 512/core across 8 cores; Wqk/Wov replicated;
each core sees all keys (full x) and computes its row block end-to-end.

Per-core key blocks are permuted host-side so the causal structure is
core-independent: slot 0 = the diagonal 512-block (local triangular mask,
generated on device), slots 1..7 = the remaining blocks, with a per-core
additive bias beta in {0, -1e30} marking fully-visible / fully-masked
blocks. This keeps one SPMD program valid for every core.

Matmul precision: float32r (fp32 with 11-bit mantissa, exact fp32
accumulation) for q/scores/AV/Wov matmuls; softmax row max subtracted in
fp32; attn stored bf16 for the DMA-xbar transposes and AV matmul.
"""

import sys

sys.path.insert(0, "/opt/trn_rl_repo")

import numpy as np
import ml_dtypes

import concourse.tile as tile
from concourse import bacc, mybir
from concourse.bass_utils import run_bass_kernel_spmd

T = 4096
D = 1024
NCORES = 8
RQ = T // NCORES  # 512 query rows per core
NKB = T // 512  # 8 key slots of 512
KC = D // 128  # 8 contraction chunks
NMT = RQ // 128  # 4 query-row tiles per core
NEG = -1.0e30

f32 = mybir.dt.float32
f32r = mybir.dt.float32r
bf16 = mybir.dt.bfloat16


def _round_f32r(a: np.ndarray) -> np.ndarray:
    """Round fp32 to f32r encoding: RNE to 11 explicit mantissa bits."""
    u = np.ascontiguousarray(a, np.float32).view(np.uint32).astype(np.uint64)
    u = (u + 0x7FF + ((u >> 12) & 1)) & ~np.uint64(0xFFF)
    return u.astype(np.uint32).view(np.float32)


def _build_nc():
    nc = bacc.Bacc(
        "TRN2", target_bir_lowering=False, debug=False, num_devices=NCORES
    )

    xqt_d = nc.dram_tensor("xqt", [D, RQ], f32r, kind="ExternalInput").ap()
    xtp_d = nc.dram_tensor("xtp", [D, T], f32r, kind="ExternalInput").ap()
    xp_d = nc.dram_tensor("xp", [T, D], bf16, kind="ExternalInput").ap()
    wqk_d = nc.dram_tensor("wqk", [D, D], f32r, kind="ExternalInput").ap()
    wov_d = nc.dram_tensor("wov", [D, D], f32r, kind="ExternalInput").ap()
    beta_d = nc.dram_tensor("beta", [128, NKB], f32, kind="ExternalInput").ap()
    out_d = nc.dram_tensor("out", [RQ, D], f32, kind="ExternalOutput").ap()

    with tile.TileContext(nc) as tc:
        # stack allocator: allocate in order of decreasing lifetime
        consts = tc.alloc_tile_pool(name="consts", bufs=1)
        o1_pool = tc.alloc_tile_pool(name="o1pool", bufs=1)
        pt_pool = tc.alloc_tile_pool(name="ptpool", bufs=1)
        p_pool = tc.alloc_tile_pool(name="ppool", bufs=6)
        s_pool = tc.alloc_tile_pool(name="spool", bufs=NMT)
        qt_pool = tc.alloc_tile_pool(name="qt", bufs=1)
        xstream = tc.alloc_tile_pool(name="xstream", bufs=4)
        wqk_pool = tc.alloc_tile_pool(name="wqkp", bufs=1)

        # constants: stats scratch, tri mask, beta
        smalls = consts.tile([128, 68], f32, name="smalls")
        beta_sb = smalls[:, 0:NKB]
        nc.sync.dma_start(beta_sb, beta_d)
        tri = consts.tile([128, NMT * 512], bf16, name="tri")
        for mt in range(NMT):
            tm = tri[:, mt * 512 : (mt + 1) * 512]
            nc.gpsimd.memset(tm, 0.0)
            # keep 0 where (mt*128 + p - y) >= 0 i.e. key y <= local row; else -1e30
            nc.gpsimd.affine_select(
                out=tm,
                in_=tm,
                compare_op=mybir.AluOpType.is_ge,
                fill=NEG,
                base=mt * 128,
                pattern=[[-1, 512]],
                channel_multiplier=1,
            )
        negmax = smalls[:, 8:12]
        lsum = smalls[:, 12:16]
        recip = smalls[:, 16:20]
        mpart = smalls[:, 20:52]
        lq = smalls[:, 52:68]

        # ---- Phase A: qT = (xq @ Wqk)^T  -> [D, RQ] in f32r --------------
        xqt_sb = wqk_pool.tile([128, KC * RQ], f32r, name="xqt_sb")
        nc.sync.dma_start(
            xqt_sb.rearrange("p (kc n) -> p kc n", kc=KC),
            xqt_d.rearrange("(kc p) n -> p kc n", p=128),
        )
        qt_sb = qt_pool.tile([128, KC * RQ], f32r, name="qt_sb")

        with (
            tc.tile_pool(name="wqkstream", bufs=3) as wqkstream,
            tc.tile_pool(name="psA", bufs=2, space="PSUM") as psA,
        ):
            for mtd in range(KC):
                wqk_blk = wqkstream.tile([128, KC * 128], f32r, name="wqk_blk", tag="wq")
                nc.sync.dma_start(
                    wqk_blk.rearrange("p (kc n) -> p kc n", kc=KC),
                    wqk_d[:, mtd * 128 : (mtd + 1) * 128].rearrange(
                        "(kc p) n -> p kc n", p=128
                    ),
                )
                ps = psA.tile([128, RQ], f32, name="ps_qt")
                for kc in range(KC):
                    nc.tensor.matmul(
                        ps[:],
                        wqk_blk[:, kc * 128 : (kc + 1) * 128],
                        xqt_sb[:, kc * RQ : (kc + 1) * RQ],
                        start=(kc == 0),
                        stop=(kc == KC - 1),
                    )
                nc.vector.tensor_copy(qt_sb[:, mtd * RQ : (mtd + 1) * RQ], ps[:])
        wqk_pool.release()

        # ---- Phase B: scores S[mt] = qT^T @ xtp + mask -------------------
        s_tiles = [s_pool.tile([128, T], f32, name=f"s_mt{mt}", tag="s") for mt in range(NMT)]
        with tc.tile_pool(name="psB", bufs=2, space="PSUM") as psB:
            for kb in range(NKB):
                halves = []
                for hh in range(2):
                    xt_h = xstream.tile(
                        [128, (KC // 2) * 512], f32r, name="xt_h", tag="xt"
                    )
                    nc.sync.dma_start(
                        xt_h.rearrange("p (kc n) -> p kc n", kc=KC // 2),
                        xtp_d[
                            hh * (D // 2) : (hh + 1) * (D // 2),
                            kb * 512 : (kb + 1) * 512,
                        ].rearrange("(kc p) n -> p kc n", p=128),
                    )
                    halves.append(xt_h)
                for mt in range(NMT):
                    ps = psB.tile([128, 512], f32, name="ps_s")
                    for kc in range(KC):
                        nc.tensor.matmul(
                            ps[:],
                            qt_sb[:, kc * RQ + mt * 128 : kc * RQ + (mt + 1) * 128],
                            halves[kc // 4][:, (kc % 4) * 512 : (kc % 4 + 1) * 512],
                            start=(kc == 0),
                            stop=(kc == KC - 1),
                        )
                    dst = s_tiles[mt][:, kb * 512 : (kb + 1) * 512]
                    if kb == 0:
                        nc.vector.tensor_add(
                            dst, ps[:], tri[:, mt * 512 : (mt + 1) * 512]
                        )
                    else:
                        nc.vector.tensor_scalar_add(
                            dst, ps[:], beta_sb[:, kb : kb + 1]
                        )
                    nc.vector.tensor_reduce(
                        mpart[:, mt * NKB + kb : mt * NKB + kb + 1],
                        dst,
                        axis=mybir.AxisListType.X,
                        op=mybir.AluOpType.max,
                    )

            # ---- Phase C: finalize rowmax per mt (negated for exp bias) ---
            for mt in range(NMT):
                nc.vector.tensor_reduce(
                    negmax[:, mt : mt + 1],
                    mpart[:, mt * NKB : (mt + 1) * NKB],
                    axis=mybir.AxisListType.X,
                    op=mybir.AluOpType.max,
                    negate=True,
                )
        xstream.release()
        qt_pool.release()

        # ---- Phase C/D: exp in quarter chunks, pipelined with DMA xbar ---
        # transposes (out[p, kc, m] = in[m, kc*128 + p]) on the ACT HWDGE
        # ring, which must carry ONLY transposes: mixing plain copies onto
        # it corrupts transfers on this stack (hw xbar-mode hazard).
        QW = T // 4  # 1024 cols per exp/transpose chunk
        pt_tiles = [
            pt_pool.tile([128, 8 * RQ], bf16, name=f"pt_q{qq}", tag=f"ptq{qq}")
            for qq in range(4)
        ]
        pt_vs = [
            ptq.rearrange("p (kc four m) -> p kc four m", kc=8, four=NMT)
            for ptq in pt_tiles
        ]
        for qq in range(4):
            for mt in range(NMT):
                p_q = p_pool.tile([128, QW], bf16, name="p_q", tag="pq")
                nc.scalar.activation(
                    p_q[:],
                    s_tiles[mt][:, qq * QW : (qq + 1) * QW],
                    mybir.ActivationFunctionType.Exp,
                    bias=negmax[:, mt : mt + 1],
                    scale=1.0,
                    accum_out=lq[:, mt * 4 + qq : mt * 4 + qq + 1],
                )
                nc.scalar.dma_start_transpose(
                    pt_vs[qq][:, :, mt, :], p_q[:]
                )
        for mt in range(NMT):
            nc.vector.tensor_reduce(
                lsum[:, mt : mt + 1],
                lq[:, mt * 4 : (mt + 1) * 4],
                axis=mybir.AxisListType.X,
                op=mybir.AluOpType.add,
            )
            nc.vector.reciprocal(recip[:, mt : mt + 1], lsum[:, mt : mt + 1])
        s_pool.release()
        p_pool.release()
        wovstream = tc.alloc_tile_pool(name="wovstream", bufs=2)

        # ---- Phase E: o1T = xp^T @ attn^T  -> [D, RQ] f32r ---------------
        o1t_sb = o1_pool.tile([128, KC * RQ], f32r, name="o1t_sb")
        with (
            tc.tile_pool(name="xpstream", bufs=3) as xpstream,
            tc.tile_pool(name="psE", bufs=2, space="PSUM") as psE,
        ):
            for mtd in range(KC):
                xpb = xpstream.tile([128, (T // 128) * 128], bf16, name="xp_blk", tag="xp")
                nc.sync.dma_start(
                    xpb.rearrange("p (kc n) -> p kc n", kc=T // 128),
                    xp_d[:, mtd * 128 : (mtd + 1) * 128].rearrange(
                        "(kc p) n -> p kc n", p=128
                    ),
                )
                ps = psE.tile([128, RQ], f32, name="ps_av")
                for kc in range(T // 128):
                    nc.tensor.matmul(
                        ps[:],
                        xpb[:, kc * 128 : (kc + 1) * 128],
                        pt_tiles[kc // 8][:, (kc % 8) * RQ : (kc % 8 + 1) * RQ],
                        start=(kc == 0),
                        stop=(kc == T // 128 - 1),
                    )
                nc.vector.tensor_copy(o1t_sb[:, mtd * RQ : (mtd + 1) * RQ], ps[:])

        # ---- Phase F: out = (o1 @ Wov) * recip ---------------------------
        with (
            tc.tile_pool(name="psF", bufs=2, space="PSUM") as psF,
            tc.tile_pool(name="outp", bufs=3) as outp,
        ):
            for nb in range(2):
                wov_blk = wovstream.tile([128, KC * 512], f32r, name="wov_blk", tag="wv")
                nc.sync.dma_start(
                    wov_blk.rearrange("p (kc n) -> p kc n", kc=KC),
                    wov_d[:, nb * 512 : (nb + 1) * 512].rearrange(
                        "(kc p) n -> p kc n", p=128
                    ),
                )
                for mt in range(NMT):
                    ps = psF.tile([128, 512], f32, name="ps_o")
                    for kc in range(KC):
                        nc.tensor.matmul(
                            ps[:],
                            o1t_sb[:, kc * RQ + mt * 128 : kc * RQ + (mt + 1) * 128],
                            wov_blk[:, kc * 512 : (kc + 1) * 512],
                            start=(kc == 0),
                            stop=(kc == KC - 1),
                        )
                    ob = outp.tile([128, 512], f32, name="ob")
                    nc.scalar.activation(
                        ob[:],
                        ps[:],
                        mybir.ActivationFunctionType.Copy,
                        bias=0.0,
                        scale=recip[:, mt : mt + 1],
                    )
                    nc.sync.dma_start(
                        out_d[mt * 128 : (mt + 1) * 128, nb * 512 : (nb + 1) * 512],
                        ob[:],
                    )

        wovstream.release()
        pt_pool.release()
        o1_pool.release()
        consts.release()

    nc.compile()
    return nc


_NC_CACHE = {}


def _get_nc():
    if "nc" not in _NC_CACHE:
        _NC_CACHE["nc"] = _build_nc()
    return _NC_CACHE["nc"]


def _prep_in_maps(x, Wqk, Wov):
    x = np.ascontiguousarray(np.asarray(x), dtype=np.float32)
    Wqk = np.ascontiguousarray(np.asarray(Wqk), dtype=np.float32)
    Wov = np.ascontiguousarray(np.asarray(Wov), dtype=np.float32)
    xT = np.ascontiguousarray(x.T)
    wqk_r = _round_f32r(Wqk)
    wov_r = _round_f32r(Wov)
    xT_r = _round_f32r(xT)  # [D, T]
    x_bf = x.astype(ml_dtypes.bfloat16)

    in_maps = []
    for c in range(NCORES):
        order = [c] + [b for b in range(NKB) if b != c]
        beta_row = np.zeros(NKB, np.float32)
        for slot, b in enumerate(order):
            if b > c:
                beta_row[slot] = NEG
        xqt = _round_f32r(xT[:, c * RQ : (c + 1) * RQ])
        xtp = np.concatenate(
            [xT_r[:, b * 512 : (b + 1) * 512] for b in order], axis=1
        )
        xp = np.concatenate([x_bf[b * 512 : (b + 1) * 512, :] for b in order], axis=0)
        in_maps.append(
            {
                "xqt": np.ascontiguousarray(xqt),
                "xtp": np.ascontiguousarray(xtp),
                "xp": np.ascontiguousarray(xp),
                "wqk": wqk_r,
                "wov": wov_r,
                "beta": np.ascontiguousarray(
                    np.broadcast_to(beta_row, (128, NKB))
                ).astype(np.float32),
            }
        )
    return in_maps


def run(x, Wqk, Wov, **spmd_kwargs):
    """Full pipeline; returns (output [T, D] fp32, BassKernelResults)."""
    import time

    nc = _get_nc()
    in_maps = _prep_in_maps(x, Wqk, Wov)
    try:
        res = run_bass_kernel_spmd(
            nc, in_maps, core_ids=list(range(NCORES)), **spmd_kwargs
        )
    except Exception:
        # a prior crashed execution can leave a core transiently
        # unrecoverable; the runtime resets it — retry once
        time.sleep(10)
        res = run_bass_kernel_spmd(
            nc, in_maps, core_ids=list(range(NCORES)), **spmd_kwargs
        )
    out = np.concatenate([res.results[c]["out"] for c in range(NCORES)], axis=0)
    return np.ascontiguousarray(out, dtype=np.float32), res


def kernel(x, Wqk, Wov):
    out, _ = run(x, Wqk, Wov)
    return out
